# revision 2
# baseline (speedup 1.0000x reference)
"""CantorGlobalAttention Trainium2 kernel, v2.

Sharding: 8 cores = 2 batches x 4 head-pairs. Each core computes Q/K/V for
its 2 heads over the full sequence (no redundant compute), cell-sparse
masked attention, and a PARTIAL output projection (contraction over its
128 of 512 hidden dims). The host sums the 4 partial projections per
batch and adds b_proj.

Attention is dense-masked but block-sparse: for each 64-query tile only
the occupied 128-key cells are processed, and within each (qtile, cell)
only the contiguous query subrange that routes there (the "extent").
Multiplicity mask M (host-built from routes) reproduces the routed
softmax exactly: exp(scale*scores) * M, denominator from a ones column
appended to V.  All matmul inputs bf16 (f32 PSUM accumulation).
"""

import sys

try:
    import concourse.bass as bass  # noqa: F401
except Exception:  # pragma: no cover
    sys.path.insert(0, "/opt/trn_rl_repo")

import numpy as np
import ml_dtypes

import concourse.bass as bass
import concourse.mybir as mybir
import concourse.tile as tile
from concourse.bass_utils import run_bass_kernel_spmd
from concourse.vector_clock import ScopedClock

dt = mybir.dt
AF = mybir.ActivationFunctionType

S = 2048
D = 512
H = 8
HD = 64
B = 2
NCORES = 8
QS = 64              # queries per attention tile
NQT = S // QS        # 32 query tiles (global; identical on all cores)
NCELL = S // 128     # 16 key cells
SCALE = HD ** -0.5   # 0.125
SC_CAP = 896         # 14 slots x 64 (max 14 occupied cells per qtile)


# ---------------------------------------------------------------------------
# walrus workaround: this walrus build accepts at most ONE sync-wait command
# per instruction; hoist extras onto same-engine nop carriers.
# ---------------------------------------------------------------------------
def _patched_drain_and_barrier(self, tick_clock, wait_clock):
    nc = self.nc
    drain_inst = nc.sync.drain()
    wait_clock.add_sem_waits(
        drain_inst.ins, ScopedClock({None: tick_clock.global_clock})
    )
    nc.all_engine_barrier()
    assert self.sems is not None
    popped = nc._tile_sem_poison_stack.pop()
    assert popped is self._sem_poison
    nc.clear_and_free_semaphores(list(self.sems.allocated().values()))
    nc.all_engine_barrier()


tile.TileContext._drain_and_barrier = _patched_drain_and_barrier


def _split_sync_waits(nc, maxw=1):
    n_fixed = 0
    for fn in nc.m.functions:
        for bb in fn.blocks:
            src = list(bb.instructions)
            out = []
            for inst in src:
                si = inst.sync_info
                waits = list(si.on_wait) if si is not None and si.on_wait else []
                if len(waits) > maxw:
                    keep = waits[-maxw:]
                    carry = waits[:-maxw]
                    for j in range(0, len(carry), maxw):
                        nop = nc.engines[inst.engine].nop(nofuse=True)
                        nc.cur_bb.bb.instructions.remove(nop.ins)
                        nop.ins.sync_info = mybir.SyncInfo(
                            on_wait=list(carry[j : j + maxw]), on_update=[]
                        )
                        out.append(nop.ins)
                    si.on_wait = keep
                    n_fixed += 1
                out.append(inst)
            bb.instructions[:] = out
    return n_fixed


# ---------------------------------------------------------------------------
# structure derived from routes (host, at runtime; identical for all cores)
# ---------------------------------------------------------------------------
def _build_struct(routes):
    r = np.clip(np.asarray(routes)[:S].astype(np.int64), 0, S - 1)
    qtiles = []          # per qt: (cells list with own first, base slot index)
    tot = 0              # total slots
    for qt in range(NQT):
        own = (qt * QS) // 128
        cells = set()
        for qq in range(QS):
            cells.update((r[qt * QS + qq] // 128).tolist())
        assert own in cells
        order = [own] + sorted(c for c in cells if c != own)
        qtiles.append((order, tot))
        tot += len(order)
        assert len(order) * QS <= SC_CAP
    # multiplicity mask, 64-wide slot blocks:
    # mt[p, (base+slot)*QS + qq] = count of cell*128+p in routes[qt*QS+qq]
    mt = np.zeros((128, tot * QS), dtype=np.float32)
    for qt, (order, base) in enumerate(qtiles):
        for si, c in enumerate(order):
            for qq in range(QS):
                row = r[qt * QS + qq]
                js = row[(row // 128) == c]
                np.add.at(mt[:, (base + si) * QS + qq], js - c * 128, 1.0)
    return qtiles, mt.astype(ml_dtypes.bfloat16), tot * QS


# ---------------------------------------------------------------------------
# device program (identical on all 8 cores; per-core data differs)
# ---------------------------------------------------------------------------
def _build_nc(qtiles, tot, reps=1):
    nc = bass.Bass("TRN2", target_bir_lowering=False, debug=False,
                   num_devices=NCORES)
    f32, bf16 = dt.float32, dt.bfloat16

    xt_d = nc.declare_dram_parameter("xt", [128, 4, S], bf16, isOutput=False)
    wqc_d = nc.declare_dram_parameter("wqc", [128, 4, 384], bf16, isOutput=False)
    wpc_d = nc.declare_dram_parameter("wpc", [128, D], bf16, isOutput=False)
    mt_d = nc.declare_dram_parameter("mt", [128, tot], bf16, isOutput=False)
    bqk_d = nc.declare_dram_parameter("bqk", [1, 128], bf16, isOutput=False)
    bqq_d = nc.declare_dram_parameter("bqq", [1, 128], bf16, isOutput=False)
    bvc_d = nc.declare_dram_parameter("bvc", [1, 128], bf16, isOutput=False)
    ones_d = nc.declare_dram_parameter("ones", [1, D], bf16, isOutput=False)
    outp = nc.declare_dram_parameter("outp", [S, D], bf16, isOutput=True)

    with tile.TileContext(nc) as tc:
        with (
            tc.tile_pool(name="const", bufs=1) as constp,
            tc.tile_pool(name="kq", bufs=1) as kqp,
            tc.tile_pool(name="at", bufs=3) as atp,
            tc.tile_pool(name="norm", bufs=3) as normp,
            tc.tile_pool(name="ps", bufs=2, space="PSUM") as psp,
            tc.tile_pool(name="sc", bufs=2, space="PSUM") as scp,
            tc.tile_pool(name="av", bufs=2, space="PSUM") as avp,
            tc.tile_pool(name="dram", bufs=2, space="DRAM") as drp,
        ):
          for rep in range(reps):
            # ---- resident loads ----
            wqc = constp.tile([128, 4, 384], bf16, tag="wqc")
            nc.sync.dma_start(out=wqc[:], in_=wqc_d[:])
            ones = constp.tile([1, D], bf16, tag="ones")
            nc.sync.dma_start(out=ones[:], in_=ones_d[:])
            bqk = constp.tile([1, 128], bf16, tag="bqk")
            nc.sync.dma_start(out=bqk[:], in_=bqk_d[:])
            bqq = constp.tile([1, 128], bf16, tag="bqq")
            nc.sync.dma_start(out=bqq[:], in_=bqq_d[:])
            bvc = constp.tile([1, 128], bf16, tag="bvc")
            nc.sync.dma_start(out=bvc[:], in_=bvc_d[:])
            xt = constp.tile([128, 4, S], bf16, tag="xt")
            for dtile in range(4):
                nc.sync.dma_start(out=xt[:, dtile, :], in_=xt_d[:, dtile, :])
            wpc = constp.tile([128, D], bf16, tag="wpc")
            nc.sync.dma_start(out=wpc[:], in_=wpc_d[:])
            mt = constp.tile([128, tot], bf16, tag="mt")
            q4 = tot // 4
            nc.scalar.dma_start(out=mt[:, 0:q4], in_=mt_d[:, 0:q4])
            nc.gpsimd.dma_start(out=mt[:, q4:2 * q4], in_=mt_d[:, q4:2 * q4])
            nc.scalar.dma_start(out=mt[:, 2 * q4:3 * q4], in_=mt_d[:, 2 * q4:3 * q4])
            nc.gpsimd.dma_start(out=mt[:, 3 * q4:tot], in_=mt_d[:, 3 * q4:tot])

            # ---- K, Q for the pair's 2 heads: [64 hd, 2 heads, S] bf16 ----
            kt = kqp.tile([64, 2, S], bf16, tag="kt", name=f"kt{rep}")
            qt_sb = kqp.tile([64, 2, S], bf16, tag="qt", name=f"qt{rep}")
            for dst, wof, brow in ((kt, 128, bqk), (qt_sb, 0, bqq)):
                for jb in range(4):
                    ps = psp.tile([128, 512], f32, tag="ps")
                    for dtile in range(4):
                        nc.tensor.matmul(
                            ps[:],
                            wqc[:, dtile, wof:wof + 128],
                            xt[:, dtile, jb * 512:(jb + 1) * 512],
                            start=(dtile == 0), stop=False,
                        )
                    nc.tensor.matmul(ps[:], brow[:], ones[:],
                                     start=False, stop=True)
                    nc.vector.tensor_copy(
                        dst[:, 0, jb * 512:(jb + 1) * 512], ps[0:64, :])
                    nc.vector.tensor_copy(
                        dst[:, 1, jb * 512:(jb + 1) * 512], ps[64:128, :])

            # ---- V (+ ones column) : [128 j, cell, head, 65] bf16 ----
            v_aug = kqp.tile([128, NCELL, 2, HD + 1], bf16, tag="vaug",
                             name=f"va{rep}")
            nc.vector.memset(v_aug[:, :, :, HD:HD + 1], 1.0)
            for cell in range(NCELL):
                vpf = psp.tile([128, 512], f32, tag="ps")
                vp = vpf[:, 0:128]
                for dtile in range(4):
                    nc.tensor.matmul(
                        vp,
                        xt[:, dtile, cell * 128:(cell + 1) * 128],
                        wqc[:, dtile, 256:384],
                        start=(dtile == 0), stop=False,
                    )
                nc.tensor.matmul(vp, ones[:, 0:128], bvc[:],
                                 start=False, stop=True)
                nc.vector.tensor_copy(
                    v_aug[:, cell, :, 0:HD],
                    vp.rearrange("p (h e) -> p h e", e=HD),
                )

            # ---- attention: per (head, qtile) over occupied cells ----
            ao = kqp.tile([128, S], bf16, tag="ao", name=f"ao{rep}")
            for h in range(2):
                for qt in range(NQT):
                    order, b0 = qtiles[qt]
                    ns = len(order)
                    qext = ns * QS
                    sc = scp.tile([128, SC_CAP], f32, tag="sc")
                    for si, c in enumerate(order):
                        nc.tensor.matmul(
                            sc[:, si * QS:(si + 1) * QS],
                            kt[:, h, c * 128:(c + 1) * 128],
                            qt_sb[:, h, qt * QS:(qt + 1) * QS],
                            start=True, stop=True,
                        )
                    at = atp.tile([128, SC_CAP], bf16, tag="at")
                    nc.scalar.activation(at[:, 0:qext], sc[:, 0:qext],
                                         AF.Exp, scale=SCALE)
                    atm = atp.tile([128, SC_CAP], bf16, tag="atm")
                    nc.vector.tensor_mul(atm[:, 0:qext], at[:, 0:qext],
                                         mt[:, b0 * QS:b0 * QS + qext])
                    av = avp.tile([HD + 1, QS], f32, tag="av")
                    for si, c in enumerate(order):
                        nc.tensor.matmul(
                            av[:],
                            v_aug[:, c, h, :],
                            atm[:, si * QS:(si + 1) * QS],
                            start=(si == 0), stop=(si == ns - 1),
                        )
                    rec = normp.tile([1, QS], f32, tag="rec")
                    nc.vector.reciprocal(rec[:], av[HD:HD + 1, :])
                    scr = drp.tile([1, QS], f32, tag="scr")
                    nc.sync.dma_start(out=scr[:], in_=rec[:])
                    sap = scr[:]
                    bcast = bass.AP(tensor=sap.tensor, offset=sap.offset,
                                    ap=[[0, 64]] + sap.ap[1:])
                    rb = normp.tile([64, QS], f32, tag="rb")
                    nc.gpsimd.dma_start(out=rb[:], in_=bcast)
                    nc.vector.tensor_mul(
                        ao[h * 64:(h + 1) * 64, qt * QS:(qt + 1) * QS],
                        av[0:HD, :], rb[:],
                    )

            # ---- partial projection (contract = this pair's 128 dims) ----
            for st in range(16):
                pp = psp.tile([128, D], f32, tag="ps")
                nc.tensor.matmul(
                    pp[:], ao[:, st * 128:(st + 1) * 128], wpc[:],
                    start=True, stop=True,
                )
                po = normp.tile([128, D], bf16, tag="po")
                nc.vector.tensor_copy(po[:], pp[:])
                nc.sync.dma_start(out=outp[st * 128:(st + 1) * 128, :], in_=po[:])

    _split_sync_waits(nc)
    return nc


_NC_CACHE = {}
_STRUCT = None


def _get_nc(reps=1):
    assert _STRUCT is not None, "_prep_inputs must run first"
    qtiles, _, tot = _STRUCT
    key = (reps, tot)
    if key not in _NC_CACHE:
        _NC_CACHE[key] = _build_nc(qtiles, tot, reps)
    return _NC_CACHE[key]


# ---------------------------------------------------------------------------
# host wrapper
# ---------------------------------------------------------------------------
def _prep_inputs(x, routes, w_qkv, b_qkv, w_proj, b_proj):
    global _STRUCT
    x = np.asarray(x, dtype=np.float32)
    w_qkv = np.asarray(w_qkv, dtype=np.float32)
    b_qkv = np.asarray(b_qkv, dtype=np.float32)
    w_proj = np.asarray(w_proj, dtype=np.float32)

    _STRUCT = _build_struct(routes)
    _, mt, tot = _STRUCT

    bf = ml_dtypes.bfloat16
    # xt[p, dt, j] = x[b, j, dt*128+p]
    xts = [
        np.ascontiguousarray(
            x[b].T.reshape(4, 128, S).transpose(1, 0, 2)).astype(bf)
        for b in range(B)
    ]
    in_maps = []
    for c in range(NCORES):
        b, pair = c // 4, c % 4
        # wqc columns: [q(128) | k(128) | v(128)] for this pair, [p, dt, n]
        cols = np.concatenate([
            w_qkv[pair * 128:(pair + 1) * 128],
            w_qkv[D + pair * 128:D + (pair + 1) * 128],
            w_qkv[2 * D + pair * 128:2 * D + (pair + 1) * 128],
        ], axis=0)                                   # (384, 512)
        wqc = np.ascontiguousarray(
            cols.T.reshape(4, 128, 384).transpose(1, 0, 2)).astype(bf)
        wpc = np.ascontiguousarray(
            w_proj[:, pair * 128:(pair + 1) * 128].T).astype(bf)  # (128, 512)
        in_maps.append({
            "xt": xts[b], "wqc": wqc, "wpc": wpc, "mt": mt,
            "bqk": np.ascontiguousarray(
                b_qkv[None, D + pair * 128:D + (pair + 1) * 128]).astype(bf),
            "bqq": np.ascontiguousarray(
                b_qkv[None, pair * 128:(pair + 1) * 128]).astype(bf),
            "bvc": np.ascontiguousarray(
                b_qkv[None, 2 * D + pair * 128:2 * D + (pair + 1) * 128]).astype(bf),
            "ones": np.ones((1, D), dtype=bf),
        })
    return in_maps


def run_cores(in_maps, reps=1, **kwargs):
    nc = _get_nc(reps)
    return run_bass_kernel_spmd(nc, in_maps, list(range(NCORES)), **kwargs)


def kernel(x, routes, w_qkv, b_qkv, w_proj, b_proj):
    in_maps = _prep_inputs(x, routes, w_qkv, b_qkv, w_proj, b_proj)
    res = run_cores(in_maps)
    b_proj = np.asarray(b_proj, dtype=np.float32)
    out = np.empty((B, S, D), dtype=np.float32)
    for b in range(B):
        acc = np.zeros((S, D), dtype=np.float32)
        for pair in range(4):
            acc += np.asarray(res.results[4 * b + pair]["outp"],
                              dtype=np.float32)
        out[b] = acc + b_proj
    return out
